# revision 19
# baseline (speedup 1.0000x reference)
"""Deformable 1D convolution for Trainium2 (8 NeuronCores, data-parallel over batch).

Math (validated against the reference):
    p[t,k]   = clip(k + offsets[b,0,t,k], 0, 2)
    c[k,j,t] = mask[b,k,t] * relu(1 - |p[t,k] - j|)      j in {0,1,2}
    out[b,o,t] = sum_{k,j} c[k,j,t] * (W_k @ x[b])[o, t+j] + bias[o]

Kernel layout strategy:
  - PE runs "x-stationary" bf16 matmuls: lhsT = x[:, chunk+j] (c on
    partitions), rhs = all three W_k^T -> PSUM Y^T_j in [t', (k,o)] layout.
  - With t on partitions the per-position coefficients are per-partition
    scalars.  Work split per chunk:
      VectorE: 6 fused scalar_tensor_tensor terms (j=0,1) read PSUM directly,
               fp32 accumulator chain seeded with the bias tile.
      ScalarE: j=2 terms as activation-copies with per-partition scale
               (fused multiply), PSUM -> bf16 SBUF.
      GpSimd:  sums the three scaled j=2 tiles and the coefficient math.
      One VectorE add joins the two chains.
  - Chunk stride 126 with 128-wide x slices keeps +j reads inside one chunk.
  - Output is produced transposed ([t, o]); host unshard transposes back.
"""

import numpy as np
import ml_dtypes
from contextlib import ExitStack

import concourse.bass as bass
import concourse.mybir as mybir
import concourse.tile as tile
from concourse import bacc
from concourse import bass_utils

F32 = mybir.dt.float32
BF16 = mybir.dt.bfloat16
OP = mybir.AluOpType
ACTF = mybir.ActivationFunctionType

B, C, L, K = 16, 128, 4096, 3
LOUT = L - (K - 1)          # 4094
NCORES = 8
BPC = B // NCORES           # batches per core
CH = 126                    # combine chunk stride (t per chunk)
NS = -(-LOUT // CH)         # 33 chunks
LPAD = NS * CH              # 4158 padded t-length for coef staging

_CACHE = {}


def _build_program():
    if "nc" in _CACHE:
        return _CACHE["nc"]

    nc = bacc.Bacc(
        "TRN2",
        target_bir_lowering=False,
        debug=False,
        enable_asserts=False,
        num_devices=NCORES,
    )

    x_in = nc.dram_tensor("x_in", [BPC, C, L], BF16, kind="ExternalInput").ap()
    # host-prearranged coef staging: [t_local(126), (s,k)] layout
    offs = nc.dram_tensor("offs", [BPC, CH, NS * K], F32, kind="ExternalInput").ap()
    maskp = nc.dram_tensor("maskp", [BPC, CH, NS * K], F32, kind="ExternalInput").ap()
    wt = nc.dram_tensor("wt", [C, K * C], BF16, kind="ExternalInput").ap()
    btile = nc.dram_tensor("btile", [128, C], F32, kind="ExternalInput").ap()
    kcst = nc.dram_tensor("kcst", [128, NS * K], F32, kind="ExternalInput").ap()
    outT = nc.dram_tensor("outT", [BPC, LOUT, C], F32, kind="ExternalOutput").ap()

    with tile.TileContext(nc) as tc, ExitStack() as ctx:
        const_pool = ctx.enter_context(tc.tile_pool(name="const", bufs=1))
        x_pool = ctx.enter_context(tc.tile_pool(name="x", bufs=2))
        coef_pool = ctx.enter_context(tc.tile_pool(name="coef", bufs=2))
        y_pool = ctx.enter_context(tc.tile_pool(name="y", bufs=4))
        acc_pool = ctx.enter_context(tc.tile_pool(name="acc", bufs=4))
        psum_pool = ctx.enter_context(tc.tile_pool(name="ps", bufs=2, space="PSUM"))

        # ---- constants (loaded once) ----
        wt_sb = const_pool.tile([128, K * C], BF16)
        nc.sync.dma_start(wt_sb[:], wt[:])
        bt_sb = const_pool.tile([128, C], F32)
        nc.sync.dma_start(bt_sb[:], btile[:])
        kc_sb = const_pool.tile([128, NS * K], F32)
        nc.sync.dma_start(kc_sb[:], kcst[:])

        xs_all, cj_all = [], []
        for b in range(BPC):
            x_sb = x_pool.tile([128, L], BF16)
            nc.sync.dma_start(x_sb[:], x_in[b])

            offT = coef_pool.tile([128, NS * K], F32, tag="offT")
            nc.sync.dma_start(offT[0:CH], offs[b])
            mT = coef_pool.tile([128, NS * K], F32, tag="mT")
            nc.sync.dma_start(mT[0:CH], maskp[b])

            # ---- coefficients on GpSimd ----
            # hat(p-j) via relu second differences; with p in [0,2] only two
            # relus are needed: A = relu(p-1), Bq = relu(p-2):
            #   u0 = (1-p) + A ; u1 = p - 2A + Bq ; u2 = A - 2Bq ; c_j = u_j*mask
            pcl = coef_pool.tile([128, NS * K], F32, tag="pcl")
            nc.gpsimd.tensor_tensor(pcl[0:CH], offT[0:CH], kc_sb[0:CH], OP.add)
            nc.gpsimd.tensor_scalar(pcl[0:CH], pcl[0:CH], 0.0, 2.0, OP.max, OP.min)
            ra = coef_pool.tile([128, NS * K], F32, tag="ra")
            nc.gpsimd.tensor_scalar(ra[0:CH], pcl[0:CH], -1.0, 0.0, OP.add, OP.max)
            rb = coef_pool.tile([128, NS * K], F32, tag="rb")
            nc.gpsimd.tensor_scalar(rb[0:CH], pcl[0:CH], -2.0, 0.0, OP.add, OP.max)

            u0 = coef_pool.tile([128, NS * K], F32, tag="u0")
            nc.gpsimd.tensor_scalar(u0[0:CH], pcl[0:CH], -1.0, 1.0, OP.mult, OP.add)
            nc.gpsimd.tensor_tensor(u0[0:CH], u0[0:CH], ra[0:CH], OP.add)
            u1 = coef_pool.tile([128, NS * K], F32, tag="u1")
            nc.gpsimd.tensor_scalar(u1[0:CH], ra[0:CH], -2.0, None, OP.mult)
            nc.gpsimd.tensor_tensor(u1[0:CH], u1[0:CH], pcl[0:CH], OP.add)
            nc.gpsimd.tensor_tensor(u1[0:CH], u1[0:CH], rb[0:CH], OP.add)
            u2 = coef_pool.tile([128, NS * K], F32, tag="u2")
            nc.gpsimd.tensor_scalar(u2[0:CH], rb[0:CH], -2.0, None, OP.mult)
            nc.gpsimd.tensor_tensor(u2[0:CH], u2[0:CH], ra[0:CH], OP.add)
            cj = []
            for j, uj in enumerate((u0, u1, u2)):
                cjt = coef_pool.tile([128, NS * K], F32, tag=f"c{j}")
                nc.gpsimd.tensor_tensor(cjt[0:CH], uj[0:CH], mT[0:CH], OP.mult)
                cj.append(cjt)
            xs_all.append(x_sb)
            cj_all.append(cj)

        # ---- conv + combine, batches interleaved chunk by chunk ----
        for s in range(NS):
            for b in range(BPC):
                x_sb = xs_all[b]
                cj = cj_all[b]
                t0 = s * CH
                ts_ = min(CH, LOUT - t0)     # valid outputs in this chunk

                psj = []
                for j in range(3):
                    msj = min(128, L - (t0 + j))
                    ps = psum_pool.tile([128, K * C], F32, tag=f"ps{j}",
                                        bufs=3 if j < 2 else 2)
                    nc.tensor.matmul(
                        ps[0:msj, :],
                        x_sb[:, t0 + j:t0 + j + msj],
                        wt_sb[:, :],
                        start=True,
                        stop=True,
                    )
                    psj.append(ps)

                col = lambda kk, jj: cj[jj][0:ts_, s * K + kk:s * K + kk + 1]

                # chain A (VectorE): bias + 5 fused terms, fp32
                chainA = [(0, 0), (1, 0), (2, 0), (0, 1), (1, 1)]
                chainB = [(2, 1), (0, 2), (1, 2), (2, 2)]

                accA = [acc_pool.tile([128, C], F32, name="accA0", tag="accA0"),
                        acc_pool.tile([128, C], F32, name="accA1", tag="accA1")]
                prev = bt_sb
                for i, (k, j) in enumerate(chainA):
                    dst = accA[i % 2]
                    nc.vector.scalar_tensor_tensor(
                        dst[0:ts_, :],
                        psj[j][0:ts_, k * C:(k + 1) * C],
                        col(k, j),
                        prev[0:ts_, :],
                        OP.mult,
                        OP.add,
                    )
                    prev = dst

                # chain B: ScalarE scaled copies (fused multiply), GpSimd sums
                tk = []
                for i, (k, j) in enumerate(chainB):
                    t = y_pool.tile([128, C], BF16, name=f"tk{i}", tag=f"tk{i}")
                    nc.scalar.activation(
                        t[0:ts_, :],
                        psj[j][0:ts_, k * C:(k + 1) * C],
                        ACTF.Copy,
                        scale=col(k, j),
                    )
                    tk.append(t)
                b01 = acc_pool.tile([128, C], BF16, name="b01", tag="b01")
                nc.gpsimd.tensor_tensor(
                    b01[0:ts_, :], tk[0][0:ts_, :], tk[1][0:ts_, :], OP.add
                )
                b23 = acc_pool.tile([128, C], BF16, name="b23", tag="b23")
                nc.gpsimd.tensor_tensor(
                    b23[0:ts_, :], tk[2][0:ts_, :], tk[3][0:ts_, :], OP.add
                )
                b03 = acc_pool.tile([128, C], BF16, name="b03", tag="b03")
                nc.gpsimd.tensor_tensor(
                    b03[0:ts_, :], b01[0:ts_, :], b23[0:ts_, :], OP.add
                )
                acc_f = acc_pool.tile([128, C], F32, tag="accF")
                nc.gpsimd.tensor_tensor(
                    acc_f[0:ts_, :], prev[0:ts_, :], b03[0:ts_, :], OP.add
                )
                nc.sync.dma_start(outT[b, t0:t0 + ts_, :], acc_f[0:ts_, :])

    nc.compile()
    _CACHE["nc"] = nc
    return nc


def _make_in_maps(x, offsets, mask, weight, bias):
    x = np.asarray(x, dtype=np.float32)
    offsets = np.asarray(offsets, dtype=np.float32)
    mask = np.asarray(mask, dtype=np.float32)
    weight = np.asarray(weight, dtype=np.float32)
    bias = np.asarray(bias, dtype=np.float32)

    bf16 = ml_dtypes.bfloat16
    x_bf = np.ascontiguousarray(x.astype(bf16))
    # wt[c, k*C + o] = weight[o, c, k]
    wt = np.ascontiguousarray(
        weight.transpose(1, 2, 0).reshape(C, K * C).astype(bf16)
    )
    btile = np.ascontiguousarray(np.broadcast_to(bias[None, :], (128, C)))
    kc = np.tile(np.arange(K, dtype=np.float32), NS)
    kcst = np.ascontiguousarray(np.broadcast_to(kc[None, :], (128, NS * K)))

    # coef staging: [CH, (s, k)] with t = s*CH + p
    offs_pad = np.zeros((B, LPAD, K), np.float32)
    offs_pad[:, :LOUT] = offsets[:, 0]
    offs_pre = np.ascontiguousarray(
        offs_pad.reshape(B, NS, CH, K).transpose(0, 2, 1, 3).reshape(B, CH, NS * K)
    )
    mask_pad = np.zeros((B, K, LPAD), np.float32)
    mask_pad[:, :, :LOUT] = mask
    mask_pre = np.ascontiguousarray(
        mask_pad.reshape(B, K, NS, CH).transpose(0, 3, 2, 1).reshape(B, CH, NS * K)
    )

    in_maps = []
    for cid in range(NCORES):
        sl = slice(cid * BPC, (cid + 1) * BPC)
        in_maps.append({
            "x_in": np.ascontiguousarray(x_bf[sl]),
            "offs": np.ascontiguousarray(offs_pre[sl]),
            "maskp": np.ascontiguousarray(mask_pre[sl]),
            "wt": wt,
            "btile": btile,
            "kcst": kcst,
        })
    return in_maps


def kernel(x, offsets, mask, weight, bias):
    nc = _build_program()
    in_maps = _make_in_maps(x, offsets, mask, weight, bias)
    res = bass_utils.run_bass_kernel_spmd(nc, in_maps, core_ids=list(range(NCORES)))
    out = np.empty((B, C, LOUT), np.float32)
    for cid in range(NCORES):
        out[cid * BPC:(cid + 1) * BPC] = res.results[cid]["outT"].transpose(0, 2, 1)
    return out


# revision 20
# speedup vs baseline: 1.0345x; 1.0345x over previous
"""Deformable 1D convolution for Trainium2 (8 NeuronCores, data-parallel over batch).

Math (validated against the reference):
    p[t,k]   = clip(k + offsets[b,0,t,k], 0, 2)
    c[k,j,t] = mask[b,k,t] * relu(1 - |p[t,k] - j|)      j in {0,1,2}
    out[b,o,t] = sum_{k,j} c[k,j,t] * (W_k @ x[b])[o, t+j] + bias[o]

Kernel layout strategy:
  - PE runs "x-stationary" bf16 matmuls: lhsT = x[:, chunk+j] (c on
    partitions), rhs = all three W_k^T -> PSUM Y^T_j in [t', (k,o)] layout.
  - With t on partitions the per-position coefficients are per-partition
    scalars.  Work split per chunk:
      VectorE: 6 fused scalar_tensor_tensor terms (j=0,1) read PSUM directly,
               fp32 accumulator chain seeded with the bias tile.
      ScalarE: j=2 terms as activation-copies with per-partition scale
               (fused multiply), PSUM -> bf16 SBUF.
      GpSimd:  sums the three scaled j=2 tiles and the coefficient math.
      One VectorE add joins the two chains.
  - Chunk stride 126 with 128-wide x slices keeps +j reads inside one chunk.
  - Output is produced transposed ([t, o]); host unshard transposes back.
"""

import numpy as np
import ml_dtypes
from contextlib import ExitStack

import concourse.bass as bass
import concourse.mybir as mybir
import concourse.tile as tile
from concourse import bacc
from concourse import bass_utils

F32 = mybir.dt.float32
BF16 = mybir.dt.bfloat16
OP = mybir.AluOpType
ACTF = mybir.ActivationFunctionType

B, C, L, K = 16, 128, 4096, 3
LOUT = L - (K - 1)          # 4094
NCORES = 8
BPC = B // NCORES           # batches per core
CH = 126                    # combine chunk stride (t per chunk)
NS = -(-LOUT // CH)         # 33 chunks
LPAD = NS * CH              # 4158 padded t-length for coef staging

_CACHE = {}


def _build_program():
    if "nc" in _CACHE:
        return _CACHE["nc"]

    nc = bacc.Bacc(
        "TRN2",
        target_bir_lowering=False,
        debug=False,
        enable_asserts=False,
        num_devices=NCORES,
    )

    x_in = nc.dram_tensor("x_in", [BPC, C, L], BF16, kind="ExternalInput").ap()
    # host-prearranged coef staging: [t_local(126), (s,k)] layout
    offs = nc.dram_tensor("offs", [BPC, CH, NS * K], F32, kind="ExternalInput").ap()
    maskp = nc.dram_tensor("maskp", [BPC, CH, NS * K], F32, kind="ExternalInput").ap()
    wt = nc.dram_tensor("wt", [C, K * C], BF16, kind="ExternalInput").ap()
    btile = nc.dram_tensor("btile", [128, C], F32, kind="ExternalInput").ap()
    kcst = nc.dram_tensor("kcst", [128, NS * K], F32, kind="ExternalInput").ap()
    outT = nc.dram_tensor("outT", [BPC, LOUT, C], F32, kind="ExternalOutput").ap()

    with tile.TileContext(nc) as tc, ExitStack() as ctx:
        const_pool = ctx.enter_context(tc.tile_pool(name="const", bufs=1))
        x_pool = ctx.enter_context(tc.tile_pool(name="x", bufs=2))
        coef_pool = ctx.enter_context(tc.tile_pool(name="coef", bufs=2))
        y_pool = ctx.enter_context(tc.tile_pool(name="y", bufs=4))
        acc_pool = ctx.enter_context(tc.tile_pool(name="acc", bufs=4))
        psum_pool = ctx.enter_context(tc.tile_pool(name="ps", bufs=2, space="PSUM"))

        # ---- constants (loaded once) ----
        wt_sb = const_pool.tile([128, K * C], BF16)
        nc.sync.dma_start(wt_sb[:], wt[:])
        bt_sb = const_pool.tile([128, C], F32)
        nc.sync.dma_start(bt_sb[:], btile[:])
        kc_sb = const_pool.tile([128, NS * K], F32)
        nc.sync.dma_start(kc_sb[:], kcst[:])

        for b in range(BPC):
            x_sb = x_pool.tile([128, L], BF16)
            nc.sync.dma_start(x_sb[:], x_in[b])

            offT = coef_pool.tile([128, NS * K], F32, tag="offT")
            nc.sync.dma_start(offT[0:CH], offs[b])
            mT = coef_pool.tile([128, NS * K], F32, tag="mT")
            nc.sync.dma_start(mT[0:CH], maskp[b])

            # ---- coefficients on GpSimd ----
            # hat(p-j) via relu second differences; with p in [0,2] only two
            # relus are needed: A = relu(p-1), Bq = relu(p-2):
            #   u0 = (1-p) + A ; u1 = p - 2A + Bq ; u2 = A - 2Bq ; c_j = u_j*mask
            pcl = coef_pool.tile([128, NS * K], F32, tag="pcl")
            nc.gpsimd.tensor_tensor(pcl[0:CH], offT[0:CH], kc_sb[0:CH], OP.add)
            nc.gpsimd.tensor_scalar(pcl[0:CH], pcl[0:CH], 0.0, 2.0, OP.max, OP.min)
            ra = coef_pool.tile([128, NS * K], F32, tag="ra")
            nc.gpsimd.tensor_scalar(ra[0:CH], pcl[0:CH], -1.0, 0.0, OP.add, OP.max)
            rb = coef_pool.tile([128, NS * K], F32, tag="rb")
            nc.gpsimd.tensor_scalar(rb[0:CH], pcl[0:CH], -2.0, 0.0, OP.add, OP.max)

            u0 = coef_pool.tile([128, NS * K], F32, tag="u0")
            nc.gpsimd.tensor_scalar(u0[0:CH], pcl[0:CH], -1.0, 1.0, OP.mult, OP.add)
            nc.gpsimd.tensor_tensor(u0[0:CH], u0[0:CH], ra[0:CH], OP.add)
            u1 = coef_pool.tile([128, NS * K], F32, tag="u1")
            nc.gpsimd.tensor_scalar(u1[0:CH], ra[0:CH], -2.0, None, OP.mult)
            nc.gpsimd.tensor_tensor(u1[0:CH], u1[0:CH], pcl[0:CH], OP.add)
            nc.gpsimd.tensor_tensor(u1[0:CH], u1[0:CH], rb[0:CH], OP.add)
            u2 = coef_pool.tile([128, NS * K], F32, tag="u2")
            nc.gpsimd.tensor_scalar(u2[0:CH], rb[0:CH], -2.0, None, OP.mult)
            nc.gpsimd.tensor_tensor(u2[0:CH], u2[0:CH], ra[0:CH], OP.add)
            cj = []
            for j, uj in enumerate((u0, u1, u2)):
                cjt = coef_pool.tile([128, NS * K], F32, tag=f"c{j}")
                nc.gpsimd.tensor_tensor(cjt[0:CH], uj[0:CH], mT[0:CH], OP.mult)
                cj.append(cjt)

            # ---- conv + combine, chunk by chunk ----
            for s in range(NS):
                t0 = s * CH
                ts_ = min(CH, LOUT - t0)     # valid outputs in this chunk

                psj = []
                for j in range(3):
                    msj = min(128, L - (t0 + j))
                    ps = psum_pool.tile([128, K * C], F32, tag=f"ps{j}",
                                        bufs=3 if j < 2 else 2)
                    nc.tensor.matmul(
                        ps[0:msj, :],
                        x_sb[:, t0 + j:t0 + j + msj],
                        wt_sb[:, :],
                        start=True,
                        stop=True,
                    )
                    psj.append(ps)

                col = lambda kk, jj: cj[jj][0:ts_, s * K + kk:s * K + kk + 1]

                # chain A (VectorE): bias + 5 fused terms, fp32
                chainA = [(0, 0), (1, 0), (2, 0), (0, 1), (1, 1)]
                chainB = [(2, 1), (0, 2), (1, 2), (2, 2)]

                accA = [acc_pool.tile([128, C], F32, name="accA0", tag="accA0"),
                        acc_pool.tile([128, C], F32, name="accA1", tag="accA1")]
                prev = bt_sb
                for i, (k, j) in enumerate(chainA):
                    dst = accA[i % 2]
                    nc.vector.scalar_tensor_tensor(
                        dst[0:ts_, :],
                        psj[j][0:ts_, k * C:(k + 1) * C],
                        col(k, j),
                        prev[0:ts_, :],
                        OP.mult,
                        OP.add,
                    )
                    prev = dst

                # chain B: ScalarE scaled copies (fused multiply), GpSimd sums
                tk = []
                for i, (k, j) in enumerate(chainB):
                    t = y_pool.tile([128, C], BF16, name=f"tk{i}", tag=f"tk{i}")
                    nc.scalar.activation(
                        t[0:ts_, :],
                        psj[j][0:ts_, k * C:(k + 1) * C],
                        ACTF.Copy,
                        scale=col(k, j),
                    )
                    tk.append(t)
                b01 = acc_pool.tile([128, C], BF16, name="b01", tag="b01")
                nc.gpsimd.tensor_tensor(
                    b01[0:ts_, :], tk[0][0:ts_, :], tk[1][0:ts_, :], OP.add
                )
                b23 = acc_pool.tile([128, C], BF16, name="b23", tag="b23")
                nc.gpsimd.tensor_tensor(
                    b23[0:ts_, :], tk[2][0:ts_, :], tk[3][0:ts_, :], OP.add
                )
                b03 = acc_pool.tile([128, C], BF16, name="b03", tag="b03")
                nc.gpsimd.tensor_tensor(
                    b03[0:ts_, :], b01[0:ts_, :], b23[0:ts_, :], OP.add
                )
                acc_f = acc_pool.tile([128, C], F32, tag="accF")
                nc.gpsimd.tensor_tensor(
                    acc_f[0:ts_, :], prev[0:ts_, :], b03[0:ts_, :], OP.add
                )
                nc.sync.dma_start(outT[b, t0:t0 + ts_, :], acc_f[0:ts_, :])

    nc.compile()
    _CACHE["nc"] = nc
    return nc


def _make_in_maps(x, offsets, mask, weight, bias):
    x = np.asarray(x, dtype=np.float32)
    offsets = np.asarray(offsets, dtype=np.float32)
    mask = np.asarray(mask, dtype=np.float32)
    weight = np.asarray(weight, dtype=np.float32)
    bias = np.asarray(bias, dtype=np.float32)

    bf16 = ml_dtypes.bfloat16
    x_bf = np.ascontiguousarray(x.astype(bf16))
    # wt[c, k*C + o] = weight[o, c, k]
    wt = np.ascontiguousarray(
        weight.transpose(1, 2, 0).reshape(C, K * C).astype(bf16)
    )
    btile = np.ascontiguousarray(np.broadcast_to(bias[None, :], (128, C)))
    kc = np.tile(np.arange(K, dtype=np.float32), NS)
    kcst = np.ascontiguousarray(np.broadcast_to(kc[None, :], (128, NS * K)))

    # coef staging: [CH, (s, k)] with t = s*CH + p
    offs_pad = np.zeros((B, LPAD, K), np.float32)
    offs_pad[:, :LOUT] = offsets[:, 0]
    offs_pre = np.ascontiguousarray(
        offs_pad.reshape(B, NS, CH, K).transpose(0, 2, 1, 3).reshape(B, CH, NS * K)
    )
    mask_pad = np.zeros((B, K, LPAD), np.float32)
    mask_pad[:, :, :LOUT] = mask
    mask_pre = np.ascontiguousarray(
        mask_pad.reshape(B, K, NS, CH).transpose(0, 3, 2, 1).reshape(B, CH, NS * K)
    )

    in_maps = []
    for cid in range(NCORES):
        sl = slice(cid * BPC, (cid + 1) * BPC)
        in_maps.append({
            "x_in": np.ascontiguousarray(x_bf[sl]),
            "offs": np.ascontiguousarray(offs_pre[sl]),
            "maskp": np.ascontiguousarray(mask_pre[sl]),
            "wt": wt,
            "btile": btile,
            "kcst": kcst,
        })
    return in_maps


def kernel(x, offsets, mask, weight, bias):
    nc = _build_program()
    in_maps = _make_in_maps(x, offsets, mask, weight, bias)
    res = bass_utils.run_bass_kernel_spmd(nc, in_maps, core_ids=list(range(NCORES)))
    out = np.empty((B, C, LOUT), np.float32)
    for cid in range(NCORES):
        out[cid * BPC:(cid + 1) * BPC] = res.results[cid]["outT"].transpose(0, 2, 1)
    return out


# revision 21
# speedup vs baseline: 1.0699x; 1.0342x over previous
"""Deformable 1D convolution for Trainium2 (8 NeuronCores, data-parallel over batch).

Math (validated against the reference):
    p[t,k]   = clip(k + offsets[b,0,t,k], 0, 2)
    c[k,j,t] = mask[b,k,t] * relu(1 - |p[t,k] - j|)      j in {0,1,2}
    out[b,o,t] = sum_{k,j} c[k,j,t] * (W_k @ x[b])[o, t+j] + bias[o]

Kernel layout strategy:
  - PE runs "x-stationary" bf16 matmuls: lhsT = x[:, chunk+j] (c on
    partitions), rhs = all three W_k^T -> PSUM Y^T_j in [t', (k,o)] layout.
  - With t on partitions the per-position coefficients are per-partition
    scalars.  Work split per chunk:
      VectorE: 6 fused scalar_tensor_tensor terms (j=0,1) read PSUM directly,
               fp32 accumulator chain seeded with the bias tile.
      ScalarE: j=2 terms as activation-copies with per-partition scale
               (fused multiply), PSUM -> bf16 SBUF.
      GpSimd:  sums the three scaled j=2 tiles and the coefficient math.
      One VectorE add joins the two chains.
  - Chunk stride 126 with 128-wide x slices keeps +j reads inside one chunk.
  - Output is produced transposed ([t, o]); host unshard transposes back.
"""

import numpy as np
import ml_dtypes
from contextlib import ExitStack

import concourse.bass as bass
import concourse.mybir as mybir
import concourse.tile as tile
from concourse import bacc
from concourse import bass_utils

F32 = mybir.dt.float32
BF16 = mybir.dt.bfloat16
OP = mybir.AluOpType
ACTF = mybir.ActivationFunctionType

B, C, L, K = 16, 128, 4096, 3
LOUT = L - (K - 1)          # 4094
NCORES = 8
BPC = B // NCORES           # batches per core
CH = 128                    # combine chunk stride (t per chunk)
NS = -(-LOUT // CH)         # 33 chunks
LPAD = NS * CH              # 4158 padded t-length for coef staging

_CACHE = {}


def _build_program():
    if "nc" in _CACHE:
        return _CACHE["nc"]

    nc = bacc.Bacc(
        "TRN2",
        target_bir_lowering=False,
        debug=False,
        enable_asserts=False,
        num_devices=NCORES,
    )

    x_in = nc.dram_tensor("x_in", [BPC, C, L], BF16, kind="ExternalInput").ap()
    # host-prearranged coef staging: [t_local(126), (s,k)] layout
    offs = nc.dram_tensor("offs", [BPC, CH, NS * K], F32, kind="ExternalInput").ap()
    maskp = nc.dram_tensor("maskp", [BPC, CH, NS * K], F32, kind="ExternalInput").ap()
    wt = nc.dram_tensor("wt", [C, K * C], BF16, kind="ExternalInput").ap()
    btile = nc.dram_tensor("btile", [128, C], F32, kind="ExternalInput").ap()
    kcst = nc.dram_tensor("kcst", [128, NS * K], F32, kind="ExternalInput").ap()
    outT = nc.dram_tensor("outT", [BPC, LOUT, C], F32, kind="ExternalOutput").ap()

    with tile.TileContext(nc) as tc, ExitStack() as ctx:
        const_pool = ctx.enter_context(tc.tile_pool(name="const", bufs=1))
        x_pool = ctx.enter_context(tc.tile_pool(name="x", bufs=2))
        coef_pool = ctx.enter_context(tc.tile_pool(name="coef", bufs=2))
        y_pool = ctx.enter_context(tc.tile_pool(name="y", bufs=4))
        acc_pool = ctx.enter_context(tc.tile_pool(name="acc", bufs=4))
        psum_pool = ctx.enter_context(tc.tile_pool(name="ps", bufs=2, space="PSUM"))

        # ---- constants (loaded once) ----
        wt_sb = const_pool.tile([128, K * C], BF16)
        nc.sync.dma_start(wt_sb[:], wt[:])
        bt_sb = const_pool.tile([128, C], F32)
        nc.sync.dma_start(bt_sb[:], btile[:])
        kc_sb = const_pool.tile([128, NS * K], F32)
        nc.sync.dma_start(kc_sb[:], kcst[:])

        for b in range(BPC):
            x_sb = x_pool.tile([128, L], BF16)
            nc.sync.dma_start(x_sb[:], x_in[b])

            offT = coef_pool.tile([128, NS * K], F32, tag="offT")
            nc.sync.dma_start(offT[0:CH], offs[b])
            mT = coef_pool.tile([128, NS * K], F32, tag="mT")
            nc.sync.dma_start(mT[0:CH], maskp[b])

            # ---- coefficients on GpSimd ----
            # hat(p-j) via relu second differences; with p in [0,2] only two
            # relus are needed: A = relu(p-1), Bq = relu(p-2):
            #   u0 = (1-p) + A ; u1 = p - 2A + Bq ; u2 = A - 2Bq ; c_j = u_j*mask
            pcl = coef_pool.tile([128, NS * K], F32, tag="pcl")
            nc.gpsimd.tensor_tensor(pcl[0:CH], offT[0:CH], kc_sb[0:CH], OP.add)
            nc.gpsimd.tensor_scalar(pcl[0:CH], pcl[0:CH], 0.0, 2.0, OP.max, OP.min)
            ra = coef_pool.tile([128, NS * K], F32, tag="ra")
            nc.gpsimd.tensor_scalar(ra[0:CH], pcl[0:CH], -1.0, 0.0, OP.add, OP.max)
            rb = coef_pool.tile([128, NS * K], F32, tag="rb")
            nc.gpsimd.tensor_scalar(rb[0:CH], pcl[0:CH], -2.0, 0.0, OP.add, OP.max)

            u0 = coef_pool.tile([128, NS * K], F32, tag="u0")
            nc.gpsimd.tensor_scalar(u0[0:CH], pcl[0:CH], -1.0, 1.0, OP.mult, OP.add)
            nc.gpsimd.tensor_tensor(u0[0:CH], u0[0:CH], ra[0:CH], OP.add)
            u1 = coef_pool.tile([128, NS * K], F32, tag="u1")
            nc.gpsimd.tensor_scalar(u1[0:CH], ra[0:CH], -2.0, None, OP.mult)
            nc.gpsimd.tensor_tensor(u1[0:CH], u1[0:CH], pcl[0:CH], OP.add)
            nc.gpsimd.tensor_tensor(u1[0:CH], u1[0:CH], rb[0:CH], OP.add)
            u2 = coef_pool.tile([128, NS * K], F32, tag="u2")
            nc.gpsimd.tensor_scalar(u2[0:CH], rb[0:CH], -2.0, None, OP.mult)
            nc.gpsimd.tensor_tensor(u2[0:CH], u2[0:CH], ra[0:CH], OP.add)
            cj = []
            for j, uj in enumerate((u0, u1, u2)):
                cjt = coef_pool.tile([128, NS * K], F32, tag=f"c{j}")
                nc.gpsimd.tensor_tensor(cjt[0:CH], uj[0:CH], mT[0:CH], OP.mult)
                cj.append(cjt)

            # ---- conv + combine, chunk by chunk ----
            for s in range(NS):
                t0 = s * CH
                ts_ = min(CH, LOUT - t0)     # valid outputs in this chunk

                psj = []
                for j in range(3):
                    msj = min(128, L - (t0 + j))
                    ps = psum_pool.tile([128, K * C], F32, tag=f"ps{j}",
                                        bufs=3 if j < 2 else 2)
                    nc.tensor.matmul(
                        ps[0:msj, :],
                        x_sb[:, t0 + j:t0 + j + msj],
                        wt_sb[:, :],
                        start=True,
                        stop=True,
                    )
                    psj.append(ps)

                col = lambda kk, jj: cj[jj][0:ts_, s * K + kk:s * K + kk + 1]

                # chain A (VectorE): bias + 5 fused terms, fp32
                chainA = [(0, 0), (1, 0), (2, 0), (0, 1), (1, 1)]
                chainB = [(2, 1), (0, 2), (1, 2), (2, 2)]

                accA = [acc_pool.tile([128, C], F32, name="accA0", tag="accA0"),
                        acc_pool.tile([128, C], F32, name="accA1", tag="accA1")]
                prev = bt_sb
                for i, (k, j) in enumerate(chainA):
                    dst = accA[i % 2]
                    nc.vector.scalar_tensor_tensor(
                        dst[0:ts_, :],
                        psj[j][0:ts_, k * C:(k + 1) * C],
                        col(k, j),
                        prev[0:ts_, :],
                        OP.mult,
                        OP.add,
                    )
                    prev = dst

                # chain B: ScalarE scaled copies (fused multiply), GpSimd sums
                tk = []
                for i, (k, j) in enumerate(chainB):
                    t = y_pool.tile([128, C], BF16, name=f"tk{i}", tag=f"tk{i}")
                    nc.scalar.activation(
                        t[0:ts_, :],
                        psj[j][0:ts_, k * C:(k + 1) * C],
                        ACTF.Copy,
                        scale=col(k, j),
                    )
                    tk.append(t)
                b01 = acc_pool.tile([128, C], BF16, name="b01", tag="b01")
                nc.gpsimd.tensor_tensor(
                    b01[0:ts_, :], tk[0][0:ts_, :], tk[1][0:ts_, :], OP.add
                )
                b23 = acc_pool.tile([128, C], BF16, name="b23", tag="b23")
                nc.gpsimd.tensor_tensor(
                    b23[0:ts_, :], tk[2][0:ts_, :], tk[3][0:ts_, :], OP.add
                )
                b03 = acc_pool.tile([128, C], BF16, name="b03", tag="b03")
                nc.gpsimd.tensor_tensor(
                    b03[0:ts_, :], b01[0:ts_, :], b23[0:ts_, :], OP.add
                )
                acc_f = acc_pool.tile([128, C], F32, tag="accF")
                nc.gpsimd.tensor_tensor(
                    acc_f[0:ts_, :], prev[0:ts_, :], b03[0:ts_, :], OP.add
                )
                nc.sync.dma_start(outT[b, t0:t0 + ts_, :], acc_f[0:ts_, :])

    nc.compile()
    _CACHE["nc"] = nc
    return nc


def _make_in_maps(x, offsets, mask, weight, bias):
    x = np.asarray(x, dtype=np.float32)
    offsets = np.asarray(offsets, dtype=np.float32)
    mask = np.asarray(mask, dtype=np.float32)
    weight = np.asarray(weight, dtype=np.float32)
    bias = np.asarray(bias, dtype=np.float32)

    bf16 = ml_dtypes.bfloat16
    x_bf = np.ascontiguousarray(x.astype(bf16))
    # wt[c, k*C + o] = weight[o, c, k]
    wt = np.ascontiguousarray(
        weight.transpose(1, 2, 0).reshape(C, K * C).astype(bf16)
    )
    btile = np.ascontiguousarray(np.broadcast_to(bias[None, :], (128, C)))
    kc = np.tile(np.arange(K, dtype=np.float32), NS)
    kcst = np.ascontiguousarray(np.broadcast_to(kc[None, :], (128, NS * K)))

    # coef staging: [CH, (s, k)] with t = s*CH + p
    offs_pad = np.zeros((B, LPAD, K), np.float32)
    offs_pad[:, :LOUT] = offsets[:, 0]
    offs_pre = np.ascontiguousarray(
        offs_pad.reshape(B, NS, CH, K).transpose(0, 2, 1, 3).reshape(B, CH, NS * K)
    )
    mask_pad = np.zeros((B, K, LPAD), np.float32)
    mask_pad[:, :, :LOUT] = mask
    mask_pre = np.ascontiguousarray(
        mask_pad.reshape(B, K, NS, CH).transpose(0, 3, 2, 1).reshape(B, CH, NS * K)
    )

    in_maps = []
    for cid in range(NCORES):
        sl = slice(cid * BPC, (cid + 1) * BPC)
        in_maps.append({
            "x_in": np.ascontiguousarray(x_bf[sl]),
            "offs": np.ascontiguousarray(offs_pre[sl]),
            "maskp": np.ascontiguousarray(mask_pre[sl]),
            "wt": wt,
            "btile": btile,
            "kcst": kcst,
        })
    return in_maps


def kernel(x, offsets, mask, weight, bias):
    nc = _build_program()
    in_maps = _make_in_maps(x, offsets, mask, weight, bias)
    res = bass_utils.run_bass_kernel_spmd(nc, in_maps, core_ids=list(range(NCORES)))
    out = np.empty((B, C, LOUT), np.float32)
    for cid in range(NCORES):
        out[cid * BPC:(cid + 1) * BPC] = res.results[cid]["outT"].transpose(0, 2, 1)
    return out


# revision 22
# speedup vs baseline: 1.1947x; 1.1167x over previous
"""Deformable 1D convolution for Trainium2 (8 NeuronCores, data-parallel over batch).

Math (validated against the reference):
    p[t,k]   = clip(k + offsets[b,0,t,k], 0, 2)
    c[k,j,t] = mask[b,k,t] * relu(1 - |p[t,k] - j|)      j in {0,1,2}
    out[b,o,t] = sum_{k,j} c[k,j,t] * (W_k @ x[b])[o, t+j] + bias[o]

Kernel layout strategy:
  - PE runs "x-stationary" bf16 matmuls: lhsT = x[:, chunk+j] (c on
    partitions), rhs = all three W_k^T -> PSUM Y^T_j in [t', (k,o)] layout.
  - With t on partitions the per-position coefficients are per-partition
    scalars.  Work split per chunk:
      VectorE: 6 fused scalar_tensor_tensor terms (j=0,1) read PSUM directly,
               fp32 accumulator chain seeded with the bias tile.
      ScalarE: j=2 terms as activation-copies with per-partition scale
               (fused multiply), PSUM -> bf16 SBUF.
      GpSimd:  sums the three scaled j=2 tiles and the coefficient math.
      One VectorE add joins the two chains.
  - Chunk stride 126 with 128-wide x slices keeps +j reads inside one chunk.
  - Output is produced transposed ([t, o]); host unshard transposes back.
"""

import numpy as np
import ml_dtypes
from contextlib import ExitStack

import concourse.bass as bass
import concourse.mybir as mybir
import concourse.tile as tile
from concourse import bacc
from concourse import bass_utils

F32 = mybir.dt.float32
BF16 = mybir.dt.bfloat16
OP = mybir.AluOpType
ACTF = mybir.ActivationFunctionType

B, C, L, K = 16, 128, 4096, 3
LOUT = L - (K - 1)          # 4094
NCORES = 8
BPC = B // NCORES           # batches per core
CH = 128                    # combine chunk stride (t per chunk)
NS = -(-LOUT // CH)         # 33 chunks
LPAD = NS * CH              # 4158 padded t-length for coef staging

_CACHE = {}


def _build_program():
    if "nc" in _CACHE:
        return _CACHE["nc"]

    nc = bacc.Bacc(
        "TRN2",
        target_bir_lowering=False,
        debug=False,
        enable_asserts=False,
        num_devices=NCORES,
    )

    x_in = nc.dram_tensor("x_in", [BPC, C, L], BF16, kind="ExternalInput").ap()
    # host-prearranged coef staging: [t_local(126), (s,k)] layout
    offs = nc.dram_tensor("offs", [BPC, CH, NS * K], F32, kind="ExternalInput").ap()
    maskp = nc.dram_tensor("maskp", [BPC, CH, NS * K], F32, kind="ExternalInput").ap()
    wt = nc.dram_tensor("wt", [C, K * C], BF16, kind="ExternalInput").ap()
    btile = nc.dram_tensor("btile", [128, C], F32, kind="ExternalInput").ap()
    kcst = nc.dram_tensor("kcst", [128, NS * K], F32, kind="ExternalInput").ap()
    outT = nc.dram_tensor("outT", [BPC, LOUT, C], F32, kind="ExternalOutput").ap()

    with tile.TileContext(nc) as tc, ExitStack() as ctx:
        const_pool = ctx.enter_context(tc.tile_pool(name="const", bufs=1))
        x_pool = ctx.enter_context(tc.tile_pool(name="x", bufs=2))
        coef_pool = ctx.enter_context(tc.tile_pool(name="coef", bufs=2))
        y_pool = ctx.enter_context(tc.tile_pool(name="y", bufs=4))
        acc_pool = ctx.enter_context(tc.tile_pool(name="acc", bufs=4))
        psum_pool = ctx.enter_context(tc.tile_pool(name="ps", bufs=2, space="PSUM"))

        # ---- constants (loaded once) ----
        wt_sb = const_pool.tile([128, K * C], BF16)
        nc.sync.dma_start(wt_sb[:], wt[:])
        bt_sb = const_pool.tile([128, C], F32)
        nc.sync.dma_start(bt_sb[:], btile[:])
        kc_sb = const_pool.tile([128, NS * K], F32)
        nc.sync.dma_start(kc_sb[:], kcst[:])

        for b in range(BPC):
            x_sb = x_pool.tile([128, L], BF16)
            nc.sync.dma_start(x_sb[:], x_in[b])

            offT = coef_pool.tile([128, NS * K], F32, tag="offT")
            nc.sync.dma_start(offT[0:CH], offs[b])
            mT = coef_pool.tile([128, NS * K], F32, tag="mT")
            nc.sync.dma_start(mT[0:CH], maskp[b])

            # ---- coefficients on VectorE ----
            # hat(p-j) via relu second differences; with p in [0,2] only two
            # relus are needed: A = relu(p-1), Bq = relu(p-2):
            #   u0 = (1-p) + A ; u1 = p - 2A + Bq ; u2 = A - 2Bq ; c_j = u_j*mask
            pcl = coef_pool.tile([128, NS * K], F32, tag="pcl")
            nc.vector.tensor_tensor(pcl[0:CH], offT[0:CH], kc_sb[0:CH], OP.add)
            nc.vector.tensor_scalar(pcl[0:CH], pcl[0:CH], 0.0, 2.0, OP.max, OP.min)
            ra = coef_pool.tile([128, NS * K], F32, tag="ra")
            nc.vector.tensor_scalar(ra[0:CH], pcl[0:CH], -1.0, 0.0, OP.add, OP.max)
            rb = coef_pool.tile([128, NS * K], F32, tag="rb")
            nc.vector.tensor_scalar(rb[0:CH], pcl[0:CH], -2.0, 0.0, OP.add, OP.max)

            u0 = coef_pool.tile([128, NS * K], F32, tag="u0")
            nc.vector.tensor_scalar(u0[0:CH], pcl[0:CH], -1.0, 1.0, OP.mult, OP.add)
            nc.vector.tensor_tensor(u0[0:CH], u0[0:CH], ra[0:CH], OP.add)
            u1 = coef_pool.tile([128, NS * K], F32, tag="u1")
            nc.vector.tensor_scalar(u1[0:CH], ra[0:CH], -2.0, None, OP.mult)
            nc.vector.tensor_tensor(u1[0:CH], u1[0:CH], pcl[0:CH], OP.add)
            nc.vector.tensor_tensor(u1[0:CH], u1[0:CH], rb[0:CH], OP.add)
            u2 = coef_pool.tile([128, NS * K], F32, tag="u2")
            nc.vector.tensor_scalar(u2[0:CH], rb[0:CH], -2.0, None, OP.mult)
            nc.vector.tensor_tensor(u2[0:CH], u2[0:CH], ra[0:CH], OP.add)
            cj = []
            for j, uj in enumerate((u0, u1, u2)):
                cjt = coef_pool.tile([128, NS * K], F32, tag=f"c{j}")
                nc.vector.tensor_tensor(cjt[0:CH], uj[0:CH], mT[0:CH], OP.mult)
                cj.append(cjt)

            # ---- conv + combine, chunk by chunk ----
            for s in range(NS):
                t0 = s * CH
                ts_ = min(CH, LOUT - t0)     # valid outputs in this chunk

                psj = []
                for j in range(3):
                    msj = min(128, L - (t0 + j))
                    ps = psum_pool.tile([128, K * C], F32, tag=f"ps{j}",
                                        bufs=3 if j < 2 else 2)
                    nc.tensor.matmul(
                        ps[0:msj, :],
                        x_sb[:, t0 + j:t0 + j + msj],
                        wt_sb[:, :],
                        start=True,
                        stop=True,
                    )
                    psj.append(ps)

                col = lambda kk, jj: cj[jj][0:ts_, s * K + kk:s * K + kk + 1]

                # chain A (VectorE): bias + 5 fused terms, fp32
                chainA = [(0, 0), (1, 0), (2, 0), (0, 1), (1, 1)]
                chainB = [(2, 1), (0, 2), (1, 2), (2, 2)]

                accA = [acc_pool.tile([128, C], F32, name="accA0", tag="accA0"),
                        acc_pool.tile([128, C], F32, name="accA1", tag="accA1")]
                prev = bt_sb
                for i, (k, j) in enumerate(chainA):
                    dst = accA[i % 2]
                    nc.vector.scalar_tensor_tensor(
                        dst[0:ts_, :],
                        psj[j][0:ts_, k * C:(k + 1) * C],
                        col(k, j),
                        prev[0:ts_, :],
                        OP.mult,
                        OP.add,
                    )
                    prev = dst

                # chain B: ScalarE scaled copies (fused multiply), GpSimd sums
                tk = []
                for i, (k, j) in enumerate(chainB):
                    t = y_pool.tile([128, C], BF16, name=f"tk{i}", tag=f"tk{i}")
                    nc.scalar.activation(
                        t[0:ts_, :],
                        psj[j][0:ts_, k * C:(k + 1) * C],
                        ACTF.Copy,
                        scale=col(k, j),
                    )
                    tk.append(t)
                b01 = acc_pool.tile([128, C], BF16, name="b01", tag="b01")
                nc.gpsimd.tensor_tensor(
                    b01[0:ts_, :], tk[0][0:ts_, :], tk[1][0:ts_, :], OP.add
                )
                b23 = acc_pool.tile([128, C], BF16, name="b23", tag="b23")
                nc.gpsimd.tensor_tensor(
                    b23[0:ts_, :], tk[2][0:ts_, :], tk[3][0:ts_, :], OP.add
                )
                b03 = acc_pool.tile([128, C], BF16, name="b03", tag="b03")
                nc.gpsimd.tensor_tensor(
                    b03[0:ts_, :], b01[0:ts_, :], b23[0:ts_, :], OP.add
                )
                acc_f = acc_pool.tile([128, C], F32, tag="accF")
                nc.gpsimd.tensor_tensor(
                    acc_f[0:ts_, :], prev[0:ts_, :], b03[0:ts_, :], OP.add
                )
                nc.sync.dma_start(outT[b, t0:t0 + ts_, :], acc_f[0:ts_, :])

    nc.compile()
    _CACHE["nc"] = nc
    return nc


def _make_in_maps(x, offsets, mask, weight, bias):
    x = np.asarray(x, dtype=np.float32)
    offsets = np.asarray(offsets, dtype=np.float32)
    mask = np.asarray(mask, dtype=np.float32)
    weight = np.asarray(weight, dtype=np.float32)
    bias = np.asarray(bias, dtype=np.float32)

    bf16 = ml_dtypes.bfloat16
    x_bf = np.ascontiguousarray(x.astype(bf16))
    # wt[c, k*C + o] = weight[o, c, k]
    wt = np.ascontiguousarray(
        weight.transpose(1, 2, 0).reshape(C, K * C).astype(bf16)
    )
    btile = np.ascontiguousarray(np.broadcast_to(bias[None, :], (128, C)))
    kc = np.tile(np.arange(K, dtype=np.float32), NS)
    kcst = np.ascontiguousarray(np.broadcast_to(kc[None, :], (128, NS * K)))

    # coef staging: [CH, (s, k)] with t = s*CH + p
    offs_pad = np.zeros((B, LPAD, K), np.float32)
    offs_pad[:, :LOUT] = offsets[:, 0]
    offs_pre = np.ascontiguousarray(
        offs_pad.reshape(B, NS, CH, K).transpose(0, 2, 1, 3).reshape(B, CH, NS * K)
    )
    mask_pad = np.zeros((B, K, LPAD), np.float32)
    mask_pad[:, :, :LOUT] = mask
    mask_pre = np.ascontiguousarray(
        mask_pad.reshape(B, K, NS, CH).transpose(0, 3, 2, 1).reshape(B, CH, NS * K)
    )

    in_maps = []
    for cid in range(NCORES):
        sl = slice(cid * BPC, (cid + 1) * BPC)
        in_maps.append({
            "x_in": np.ascontiguousarray(x_bf[sl]),
            "offs": np.ascontiguousarray(offs_pre[sl]),
            "maskp": np.ascontiguousarray(mask_pre[sl]),
            "wt": wt,
            "btile": btile,
            "kcst": kcst,
        })
    return in_maps


def kernel(x, offsets, mask, weight, bias):
    nc = _build_program()
    in_maps = _make_in_maps(x, offsets, mask, weight, bias)
    res = bass_utils.run_bass_kernel_spmd(nc, in_maps, core_ids=list(range(NCORES)))
    out = np.empty((B, C, LOUT), np.float32)
    for cid in range(NCORES):
        out[cid * BPC:(cid + 1) * BPC] = res.results[cid]["outT"].transpose(0, 2, 1)
    return out


# revision 23
# speedup vs baseline: 1.2246x; 1.0250x over previous
"""Deformable 1D convolution for Trainium2 (8 NeuronCores, data-parallel over batch).

Math (validated against the reference):
    p[t,k]   = clip(k + offsets[b,0,t,k], 0, 2)
    c[k,j,t] = mask[b,k,t] * relu(1 - |p[t,k] - j|)      j in {0,1,2}
    out[b,o,t] = sum_{k,j} c[k,j,t] * (W_k @ x[b])[o, t+j] + bias[o]

Kernel layout strategy:
  - PE runs "x-stationary" bf16 matmuls: lhsT = x[:, chunk+j] (c on
    partitions), rhs = all three W_k^T -> PSUM Y^T_j in [t', (k,o)] layout.
  - With t on partitions the per-position coefficients are per-partition
    scalars.  Work split per chunk:
      VectorE: 6 fused scalar_tensor_tensor terms (j=0,1) read PSUM directly,
               fp32 accumulator chain seeded with the bias tile.
      ScalarE: j=2 terms as activation-copies with per-partition scale
               (fused multiply), PSUM -> bf16 SBUF.
      GpSimd:  sums the three scaled j=2 tiles and the coefficient math.
      One VectorE add joins the two chains.
  - Chunk stride 126 with 128-wide x slices keeps +j reads inside one chunk.
  - Output is produced transposed ([t, o]); host unshard transposes back.
"""

import numpy as np
import ml_dtypes
from contextlib import ExitStack

import concourse.bass as bass
import concourse.mybir as mybir
import concourse.tile as tile
from concourse import bacc
from concourse import bass_utils

F32 = mybir.dt.float32
BF16 = mybir.dt.bfloat16
OP = mybir.AluOpType
ACTF = mybir.ActivationFunctionType

B, C, L, K = 16, 128, 4096, 3
LOUT = L - (K - 1)          # 4094
NCORES = 8
BPC = B // NCORES           # batches per core
CH = 128                    # combine chunk stride (t per chunk)
NS = -(-LOUT // CH)         # 33 chunks
LPAD = NS * CH              # 4158 padded t-length for coef staging

_CACHE = {}


def _build_program():
    if "nc" in _CACHE:
        return _CACHE["nc"]

    nc = bacc.Bacc(
        "TRN2",
        target_bir_lowering=False,
        debug=False,
        enable_asserts=False,
        num_devices=NCORES,
    )

    x_in = nc.dram_tensor("x_in", [BPC, C, L], BF16, kind="ExternalInput").ap()
    # host-prearranged coef staging: [t_local(126), (s,k)] layout
    offs = nc.dram_tensor("offs", [BPC, CH, NS * K], F32, kind="ExternalInput").ap()
    maskp = nc.dram_tensor("maskp", [BPC, CH, NS * K], F32, kind="ExternalInput").ap()
    wt = nc.dram_tensor("wt", [C, K * C], BF16, kind="ExternalInput").ap()
    btile = nc.dram_tensor("btile", [128, C], F32, kind="ExternalInput").ap()
    kcst = nc.dram_tensor("kcst", [128, NS * K], F32, kind="ExternalInput").ap()
    outT = nc.dram_tensor("outT", [BPC, LOUT, C], F32, kind="ExternalOutput").ap()

    with tile.TileContext(nc) as tc, ExitStack() as ctx:
        const_pool = ctx.enter_context(tc.tile_pool(name="const", bufs=1))
        x_pool = ctx.enter_context(tc.tile_pool(name="x", bufs=2))
        coef_pool = ctx.enter_context(tc.tile_pool(name="coef", bufs=2))
        y_pool = ctx.enter_context(tc.tile_pool(name="y", bufs=6))
        acc_pool = ctx.enter_context(tc.tile_pool(name="acc", bufs=6))
        psum_pool = ctx.enter_context(tc.tile_pool(name="ps", bufs=2, space="PSUM"))

        # ---- constants (loaded once) ----
        wt_sb = const_pool.tile([128, K * C], BF16)
        nc.sync.dma_start(wt_sb[:], wt[:])
        bt_sb = const_pool.tile([128, C], F32)
        nc.sync.dma_start(bt_sb[:], btile[:])
        kc_sb = const_pool.tile([128, NS * K], F32)
        nc.sync.dma_start(kc_sb[:], kcst[:])

        for b in range(BPC):
            x_sb = x_pool.tile([128, L], BF16)
            nc.sync.dma_start(x_sb[:], x_in[b])

            offT = coef_pool.tile([128, NS * K], F32, tag="offT")
            nc.sync.dma_start(offT[0:CH], offs[b])
            mT = coef_pool.tile([128, NS * K], F32, tag="mT")
            nc.sync.dma_start(mT[0:CH], maskp[b])

            # ---- coefficients on VectorE ----
            # hat(p-j) via relu second differences; with p in [0,2] only two
            # relus are needed: A = relu(p-1), Bq = relu(p-2):
            #   u0 = (1-p) + A ; u1 = p - 2A + Bq ; u2 = A - 2Bq ; c_j = u_j*mask
            pcl = coef_pool.tile([128, NS * K], F32, tag="pcl")
            nc.vector.tensor_tensor(pcl[0:CH], offT[0:CH], kc_sb[0:CH], OP.add)
            nc.vector.tensor_scalar(pcl[0:CH], pcl[0:CH], 0.0, 2.0, OP.max, OP.min)
            ra = coef_pool.tile([128, NS * K], F32, tag="ra")
            nc.vector.tensor_scalar(ra[0:CH], pcl[0:CH], -1.0, 0.0, OP.add, OP.max)
            rb = coef_pool.tile([128, NS * K], F32, tag="rb")
            nc.vector.tensor_scalar(rb[0:CH], pcl[0:CH], -2.0, 0.0, OP.add, OP.max)

            u0 = coef_pool.tile([128, NS * K], F32, tag="u0")
            nc.vector.tensor_scalar(u0[0:CH], pcl[0:CH], -1.0, 1.0, OP.mult, OP.add)
            nc.vector.tensor_tensor(u0[0:CH], u0[0:CH], ra[0:CH], OP.add)
            u1 = coef_pool.tile([128, NS * K], F32, tag="u1")
            nc.vector.tensor_scalar(u1[0:CH], ra[0:CH], -2.0, None, OP.mult)
            nc.vector.tensor_tensor(u1[0:CH], u1[0:CH], pcl[0:CH], OP.add)
            nc.vector.tensor_tensor(u1[0:CH], u1[0:CH], rb[0:CH], OP.add)
            u2 = coef_pool.tile([128, NS * K], F32, tag="u2")
            nc.vector.tensor_scalar(u2[0:CH], rb[0:CH], -2.0, None, OP.mult)
            nc.vector.tensor_tensor(u2[0:CH], u2[0:CH], ra[0:CH], OP.add)
            cj = []
            for j, uj in enumerate((u0, u1, u2)):
                cjt = coef_pool.tile([128, NS * K], F32, tag=f"c{j}")
                nc.vector.tensor_tensor(cjt[0:CH], uj[0:CH], mT[0:CH], OP.mult)
                cj.append(cjt)

            # ---- conv + combine, chunk by chunk ----
            for s in range(NS):
                t0 = s * CH
                ts_ = min(CH, LOUT - t0)     # valid outputs in this chunk

                psj = []
                for j in range(3):
                    msj = min(128, L - (t0 + j))
                    ps = psum_pool.tile([128, K * C], F32, tag=f"ps{j}",
                                        bufs=3 if j < 2 else 2)
                    nc.tensor.matmul(
                        ps[0:msj, :],
                        x_sb[:, t0 + j:t0 + j + msj],
                        wt_sb[:, :],
                        start=True,
                        stop=True,
                    )
                    psj.append(ps)

                col = lambda kk, jj: cj[jj][0:ts_, s * K + kk:s * K + kk + 1]

                # chain A (VectorE): bias + 5 fused terms, fp32
                chainA = [(0, 0), (1, 0), (2, 0), (0, 1), (1, 1)]
                chainB = [(2, 1), (0, 2), (1, 2), (2, 2)]

                accA = [acc_pool.tile([128, C], F32, name="accA0", tag="accA0"),
                        acc_pool.tile([128, C], F32, name="accA1", tag="accA1")]
                prev = bt_sb
                for i, (k, j) in enumerate(chainA):
                    dst = accA[i % 2]
                    nc.vector.scalar_tensor_tensor(
                        dst[0:ts_, :],
                        psj[j][0:ts_, k * C:(k + 1) * C],
                        col(k, j),
                        prev[0:ts_, :],
                        OP.mult,
                        OP.add,
                    )
                    prev = dst

                # chain B: ScalarE scaled copies (fused multiply), GpSimd sums
                tk = []
                for i, (k, j) in enumerate(chainB):
                    t = y_pool.tile([128, C], BF16, name=f"tk{i}", tag=f"tk{i}")
                    nc.scalar.activation(
                        t[0:ts_, :],
                        psj[j][0:ts_, k * C:(k + 1) * C],
                        ACTF.Copy,
                        scale=col(k, j),
                    )
                    tk.append(t)
                b01 = acc_pool.tile([128, C], BF16, name="b01", tag="b01")
                nc.gpsimd.tensor_tensor(
                    b01[0:ts_, :], tk[0][0:ts_, :], tk[1][0:ts_, :], OP.add
                )
                b23 = acc_pool.tile([128, C], BF16, name="b23", tag="b23")
                nc.gpsimd.tensor_tensor(
                    b23[0:ts_, :], tk[2][0:ts_, :], tk[3][0:ts_, :], OP.add
                )
                b03 = acc_pool.tile([128, C], BF16, name="b03", tag="b03")
                nc.gpsimd.tensor_tensor(
                    b03[0:ts_, :], b01[0:ts_, :], b23[0:ts_, :], OP.add
                )
                acc_f = acc_pool.tile([128, C], F32, tag="accF")
                nc.gpsimd.tensor_tensor(
                    acc_f[0:ts_, :], prev[0:ts_, :], b03[0:ts_, :], OP.add
                )
                nc.sync.dma_start(outT[b, t0:t0 + ts_, :], acc_f[0:ts_, :])

    nc.compile()
    _CACHE["nc"] = nc
    return nc


def _make_in_maps(x, offsets, mask, weight, bias):
    x = np.asarray(x, dtype=np.float32)
    offsets = np.asarray(offsets, dtype=np.float32)
    mask = np.asarray(mask, dtype=np.float32)
    weight = np.asarray(weight, dtype=np.float32)
    bias = np.asarray(bias, dtype=np.float32)

    bf16 = ml_dtypes.bfloat16
    x_bf = np.ascontiguousarray(x.astype(bf16))
    # wt[c, k*C + o] = weight[o, c, k]
    wt = np.ascontiguousarray(
        weight.transpose(1, 2, 0).reshape(C, K * C).astype(bf16)
    )
    btile = np.ascontiguousarray(np.broadcast_to(bias[None, :], (128, C)))
    kc = np.tile(np.arange(K, dtype=np.float32), NS)
    kcst = np.ascontiguousarray(np.broadcast_to(kc[None, :], (128, NS * K)))

    # coef staging: [CH, (s, k)] with t = s*CH + p
    offs_pad = np.zeros((B, LPAD, K), np.float32)
    offs_pad[:, :LOUT] = offsets[:, 0]
    offs_pre = np.ascontiguousarray(
        offs_pad.reshape(B, NS, CH, K).transpose(0, 2, 1, 3).reshape(B, CH, NS * K)
    )
    mask_pad = np.zeros((B, K, LPAD), np.float32)
    mask_pad[:, :, :LOUT] = mask
    mask_pre = np.ascontiguousarray(
        mask_pad.reshape(B, K, NS, CH).transpose(0, 3, 2, 1).reshape(B, CH, NS * K)
    )

    in_maps = []
    for cid in range(NCORES):
        sl = slice(cid * BPC, (cid + 1) * BPC)
        in_maps.append({
            "x_in": np.ascontiguousarray(x_bf[sl]),
            "offs": np.ascontiguousarray(offs_pre[sl]),
            "maskp": np.ascontiguousarray(mask_pre[sl]),
            "wt": wt,
            "btile": btile,
            "kcst": kcst,
        })
    return in_maps


def kernel(x, offsets, mask, weight, bias):
    nc = _build_program()
    in_maps = _make_in_maps(x, offsets, mask, weight, bias)
    res = bass_utils.run_bass_kernel_spmd(nc, in_maps, core_ids=list(range(NCORES)))
    out = np.empty((B, C, LOUT), np.float32)
    for cid in range(NCORES):
        out[cid * BPC:(cid + 1) * BPC] = res.results[cid]["outT"].transpose(0, 2, 1)
    return out


# revision 24
# speedup vs baseline: 1.2283x; 1.0030x over previous
"""Deformable 1D convolution for Trainium2 (8 NeuronCores, data-parallel over batch).

Math (validated against the reference):
    p[t,k]   = clip(k + offsets[b,0,t,k], 0, 2)
    c[k,j,t] = mask[b,k,t] * relu(1 - |p[t,k] - j|)      j in {0,1,2}
    out[b,o,t] = sum_{k,j} c[k,j,t] * (W_k @ x[b])[o, t+j] + bias[o]

Kernel layout strategy:
  - PE runs "x-stationary" bf16 matmuls: lhsT = x[:, chunk+j] (c on
    partitions), rhs = all three W_k^T -> PSUM Y^T_j in [t', (k,o)] layout.
  - With t on partitions the per-position coefficients are per-partition
    scalars.  Work split per chunk:
      VectorE: 6 fused scalar_tensor_tensor terms (j=0,1) read PSUM directly,
               fp32 accumulator chain seeded with the bias tile.
      ScalarE: j=2 terms as activation-copies with per-partition scale
               (fused multiply), PSUM -> bf16 SBUF.
      GpSimd:  sums the three scaled j=2 tiles and the coefficient math.
      One VectorE add joins the two chains.
  - Chunk stride 126 with 128-wide x slices keeps +j reads inside one chunk.
  - Output is produced transposed ([t, o]); host unshard transposes back.
"""

import numpy as np
import ml_dtypes
from contextlib import ExitStack

import concourse.bass as bass
import concourse.mybir as mybir
import concourse.tile as tile
from concourse import bacc
from concourse import bass_utils

F32 = mybir.dt.float32
BF16 = mybir.dt.bfloat16
OP = mybir.AluOpType
ACTF = mybir.ActivationFunctionType

B, C, L, K = 16, 128, 4096, 3
LOUT = L - (K - 1)          # 4094
NCORES = 8
BPC = B // NCORES           # batches per core
CH = 128                    # combine chunk stride (t per chunk)
NS = -(-LOUT // CH)         # 33 chunks
LPAD = NS * CH              # 4158 padded t-length for coef staging

_CACHE = {}


def _build_program():
    if "nc" in _CACHE:
        return _CACHE["nc"]

    nc = bacc.Bacc(
        "TRN2",
        target_bir_lowering=False,
        debug=False,
        enable_asserts=False,
        num_devices=NCORES,
    )

    x_in = nc.dram_tensor("x_in", [BPC, C, L], BF16, kind="ExternalInput").ap()
    # host-prearranged coef staging: [t_local(126), (s,k)] layout
    offs = nc.dram_tensor("offs", [BPC, CH, NS * K], F32, kind="ExternalInput").ap()
    maskp = nc.dram_tensor("maskp", [BPC, CH, NS * K], F32, kind="ExternalInput").ap()
    wt = nc.dram_tensor("wt", [C, K * C], BF16, kind="ExternalInput").ap()
    btile = nc.dram_tensor("btile", [128, C], F32, kind="ExternalInput").ap()
    kcst = nc.dram_tensor("kcst", [128, NS * K], F32, kind="ExternalInput").ap()
    outT = nc.dram_tensor("outT", [BPC, LOUT, C], F32, kind="ExternalOutput").ap()

    with tile.TileContext(nc) as tc, ExitStack() as ctx:
        const_pool = ctx.enter_context(tc.tile_pool(name="const", bufs=1))
        x_pool = ctx.enter_context(tc.tile_pool(name="x", bufs=2))
        coef_pool = ctx.enter_context(tc.tile_pool(name="coef", bufs=2))
        y_pool = ctx.enter_context(tc.tile_pool(name="y", bufs=8))
        acc_pool = ctx.enter_context(tc.tile_pool(name="acc", bufs=8))
        psum_pool = ctx.enter_context(tc.tile_pool(name="ps", bufs=2, space="PSUM"))

        # ---- constants (loaded once) ----
        wt_sb = const_pool.tile([128, K * C], BF16)
        nc.sync.dma_start(wt_sb[:], wt[:])
        bt_sb = const_pool.tile([128, C], F32)
        nc.sync.dma_start(bt_sb[:], btile[:])
        kc_sb = const_pool.tile([128, NS * K], F32)
        nc.sync.dma_start(kc_sb[:], kcst[:])

        for b in range(BPC):
            x_sb = x_pool.tile([128, L], BF16)
            nc.sync.dma_start(x_sb[:], x_in[b])

            offT = coef_pool.tile([128, NS * K], F32, tag="offT")
            nc.sync.dma_start(offT[0:CH], offs[b])
            mT = coef_pool.tile([128, NS * K], F32, tag="mT")
            nc.sync.dma_start(mT[0:CH], maskp[b])

            # ---- coefficients on VectorE ----
            # hat(p-j) via relu second differences; with p in [0,2] only two
            # relus are needed: A = relu(p-1), Bq = relu(p-2):
            #   u0 = (1-p) + A ; u1 = p - 2A + Bq ; u2 = A - 2Bq ; c_j = u_j*mask
            pcl = coef_pool.tile([128, NS * K], F32, tag="pcl")
            nc.vector.tensor_tensor(pcl[0:CH], offT[0:CH], kc_sb[0:CH], OP.add)
            nc.vector.tensor_scalar(pcl[0:CH], pcl[0:CH], 0.0, 2.0, OP.max, OP.min)
            ra = coef_pool.tile([128, NS * K], F32, tag="ra")
            nc.vector.tensor_scalar(ra[0:CH], pcl[0:CH], -1.0, 0.0, OP.add, OP.max)
            rb = coef_pool.tile([128, NS * K], F32, tag="rb")
            nc.vector.tensor_scalar(rb[0:CH], pcl[0:CH], -2.0, 0.0, OP.add, OP.max)

            u0 = coef_pool.tile([128, NS * K], F32, tag="u0")
            nc.vector.tensor_scalar(u0[0:CH], pcl[0:CH], -1.0, 1.0, OP.mult, OP.add)
            nc.vector.tensor_tensor(u0[0:CH], u0[0:CH], ra[0:CH], OP.add)
            u1 = coef_pool.tile([128, NS * K], F32, tag="u1")
            nc.vector.tensor_scalar(u1[0:CH], ra[0:CH], -2.0, None, OP.mult)
            nc.vector.tensor_tensor(u1[0:CH], u1[0:CH], pcl[0:CH], OP.add)
            nc.vector.tensor_tensor(u1[0:CH], u1[0:CH], rb[0:CH], OP.add)
            u2 = coef_pool.tile([128, NS * K], F32, tag="u2")
            nc.vector.tensor_scalar(u2[0:CH], rb[0:CH], -2.0, None, OP.mult)
            nc.vector.tensor_tensor(u2[0:CH], u2[0:CH], ra[0:CH], OP.add)
            cj = []
            for j, uj in enumerate((u0, u1, u2)):
                cjt = coef_pool.tile([128, NS * K], F32, tag=f"c{j}")
                nc.vector.tensor_tensor(cjt[0:CH], uj[0:CH], mT[0:CH], OP.mult)
                cj.append(cjt)

            # ---- conv + combine, chunk by chunk ----
            for s in range(NS):
                t0 = s * CH
                ts_ = min(CH, LOUT - t0)     # valid outputs in this chunk

                psj = []
                for j in range(3):
                    msj = min(128, L - (t0 + j))
                    ps = psum_pool.tile([128, K * C], F32, tag=f"ps{j}",
                                        bufs=3 if j < 2 else 2)
                    nc.tensor.matmul(
                        ps[0:msj, :],
                        x_sb[:, t0 + j:t0 + j + msj],
                        wt_sb[:, :],
                        start=True,
                        stop=True,
                    )
                    psj.append(ps)

                col = lambda kk, jj: cj[jj][0:ts_, s * K + kk:s * K + kk + 1]

                # chain A (VectorE): bias + 5 fused terms, fp32
                chainA = [(0, 0), (1, 0), (2, 0), (0, 1), (1, 1)]
                chainB = [(2, 1), (0, 2), (1, 2), (2, 2)]

                accA = [acc_pool.tile([128, C], F32, name="accA0", tag="accA0"),
                        acc_pool.tile([128, C], F32, name="accA1", tag="accA1")]
                prev = bt_sb
                for i, (k, j) in enumerate(chainA):
                    dst = accA[i % 2]
                    nc.vector.scalar_tensor_tensor(
                        dst[0:ts_, :],
                        psj[j][0:ts_, k * C:(k + 1) * C],
                        col(k, j),
                        prev[0:ts_, :],
                        OP.mult,
                        OP.add,
                    )
                    prev = dst

                # chain B: ScalarE scaled copies (fused multiply), GpSimd sums
                tk = []
                for i, (k, j) in enumerate(chainB):
                    t = y_pool.tile([128, C], BF16, name=f"tk{i}", tag=f"tk{i}")
                    nc.scalar.activation(
                        t[0:ts_, :],
                        psj[j][0:ts_, k * C:(k + 1) * C],
                        ACTF.Copy,
                        scale=col(k, j),
                    )
                    tk.append(t)
                b01 = acc_pool.tile([128, C], BF16, name="b01", tag="b01")
                nc.gpsimd.tensor_tensor(
                    b01[0:ts_, :], tk[0][0:ts_, :], tk[1][0:ts_, :], OP.add
                )
                b23 = acc_pool.tile([128, C], BF16, name="b23", tag="b23")
                nc.gpsimd.tensor_tensor(
                    b23[0:ts_, :], tk[2][0:ts_, :], tk[3][0:ts_, :], OP.add
                )
                b03 = acc_pool.tile([128, C], BF16, name="b03", tag="b03")
                nc.gpsimd.tensor_tensor(
                    b03[0:ts_, :], b01[0:ts_, :], b23[0:ts_, :], OP.add
                )
                acc_f = acc_pool.tile([128, C], F32, tag="accF")
                nc.gpsimd.tensor_tensor(
                    acc_f[0:ts_, :], prev[0:ts_, :], b03[0:ts_, :], OP.add
                )
                nc.sync.dma_start(outT[b, t0:t0 + ts_, :], acc_f[0:ts_, :])

    nc.compile()
    _CACHE["nc"] = nc
    return nc


def _make_in_maps(x, offsets, mask, weight, bias):
    x = np.asarray(x, dtype=np.float32)
    offsets = np.asarray(offsets, dtype=np.float32)
    mask = np.asarray(mask, dtype=np.float32)
    weight = np.asarray(weight, dtype=np.float32)
    bias = np.asarray(bias, dtype=np.float32)

    bf16 = ml_dtypes.bfloat16
    x_bf = np.ascontiguousarray(x.astype(bf16))
    # wt[c, k*C + o] = weight[o, c, k]
    wt = np.ascontiguousarray(
        weight.transpose(1, 2, 0).reshape(C, K * C).astype(bf16)
    )
    btile = np.ascontiguousarray(np.broadcast_to(bias[None, :], (128, C)))
    kc = np.tile(np.arange(K, dtype=np.float32), NS)
    kcst = np.ascontiguousarray(np.broadcast_to(kc[None, :], (128, NS * K)))

    # coef staging: [CH, (s, k)] with t = s*CH + p
    offs_pad = np.zeros((B, LPAD, K), np.float32)
    offs_pad[:, :LOUT] = offsets[:, 0]
    offs_pre = np.ascontiguousarray(
        offs_pad.reshape(B, NS, CH, K).transpose(0, 2, 1, 3).reshape(B, CH, NS * K)
    )
    mask_pad = np.zeros((B, K, LPAD), np.float32)
    mask_pad[:, :, :LOUT] = mask
    mask_pre = np.ascontiguousarray(
        mask_pad.reshape(B, K, NS, CH).transpose(0, 3, 2, 1).reshape(B, CH, NS * K)
    )

    in_maps = []
    for cid in range(NCORES):
        sl = slice(cid * BPC, (cid + 1) * BPC)
        in_maps.append({
            "x_in": np.ascontiguousarray(x_bf[sl]),
            "offs": np.ascontiguousarray(offs_pre[sl]),
            "maskp": np.ascontiguousarray(mask_pre[sl]),
            "wt": wt,
            "btile": btile,
            "kcst": kcst,
        })
    return in_maps


def kernel(x, offsets, mask, weight, bias):
    nc = _build_program()
    in_maps = _make_in_maps(x, offsets, mask, weight, bias)
    res = bass_utils.run_bass_kernel_spmd(nc, in_maps, core_ids=list(range(NCORES)))
    out = np.empty((B, C, LOUT), np.float32)
    for cid in range(NCORES):
        out[cid * BPC:(cid + 1) * BPC] = res.results[cid]["outT"].transpose(0, 2, 1)
    return out


# revision 25
# speedup vs baseline: 1.2461x; 1.0146x over previous
"""Deformable 1D convolution for Trainium2 (8 NeuronCores, data-parallel over batch).

Math (validated against the reference):
    p[t,k]   = clip(k + offsets[b,0,t,k], 0, 2)
    c[k,j,t] = mask[b,k,t] * relu(1 - |p[t,k] - j|)      j in {0,1,2}
    out[b,o,t] = sum_{k,j} c[k,j,t] * (W_k @ x[b])[o, t+j] + bias[o]

Kernel layout strategy:
  - PE runs "x-stationary" bf16 matmuls: lhsT = x[:, chunk+j] (c on
    partitions), rhs = all three W_k^T -> PSUM Y^T_j in [t', (k,o)] layout.
  - With t on partitions the per-position coefficients are per-partition
    scalars.  Work split per chunk:
      VectorE: 6 fused scalar_tensor_tensor terms (j=0,1) read PSUM directly,
               fp32 accumulator chain seeded with the bias tile.
      ScalarE: j=2 terms as activation-copies with per-partition scale
               (fused multiply), PSUM -> bf16 SBUF.
      GpSimd:  sums the three scaled j=2 tiles and the coefficient math.
      One VectorE add joins the two chains.
  - Chunk stride 126 with 128-wide x slices keeps +j reads inside one chunk.
  - Output is produced transposed ([t, o]); host unshard transposes back.
"""

import numpy as np
import ml_dtypes
from contextlib import ExitStack

import concourse.bass as bass
import concourse.mybir as mybir
import concourse.tile as tile
from concourse import bacc
from concourse import bass_utils

F32 = mybir.dt.float32
BF16 = mybir.dt.bfloat16
OP = mybir.AluOpType
ACTF = mybir.ActivationFunctionType

B, C, L, K = 16, 128, 4096, 3
LOUT = L - (K - 1)          # 4094
NCORES = 8
BPC = B // NCORES           # batches per core
CH = 128                    # combine chunk stride (t per chunk)
NS = -(-LOUT // CH)         # 33 chunks
LPAD = NS * CH              # 4158 padded t-length for coef staging

_CACHE = {}


def _build_program():
    if "nc" in _CACHE:
        return _CACHE["nc"]

    nc = bacc.Bacc(
        "TRN2",
        target_bir_lowering=False,
        debug=False,
        enable_asserts=False,
        num_devices=NCORES,
    )

    x_in = nc.dram_tensor("x_in", [BPC, C, L], BF16, kind="ExternalInput").ap()
    # host-prearranged coef staging: [t_local(126), (s,k)] layout
    offs = nc.dram_tensor("offs", [BPC, CH, NS * K], F32, kind="ExternalInput").ap()
    maskp = nc.dram_tensor("maskp", [BPC, CH, NS * K], F32, kind="ExternalInput").ap()
    wt = nc.dram_tensor("wt", [C, K * C], BF16, kind="ExternalInput").ap()
    btile = nc.dram_tensor("btile", [128, C], F32, kind="ExternalInput").ap()
    kcst = nc.dram_tensor("kcst", [128, NS * K], F32, kind="ExternalInput").ap()
    outT = nc.dram_tensor("outT", [BPC, LOUT, C], F32, kind="ExternalOutput").ap()

    with tile.TileContext(nc) as tc, ExitStack() as ctx:
        const_pool = ctx.enter_context(tc.tile_pool(name="const", bufs=1))
        x_pool = ctx.enter_context(tc.tile_pool(name="x", bufs=2))
        coef_pool = ctx.enter_context(tc.tile_pool(name="coef", bufs=2))
        y_pool = ctx.enter_context(tc.tile_pool(name="y", bufs=8))
        acc_pool = ctx.enter_context(tc.tile_pool(name="acc", bufs=8))
        psum_pool = ctx.enter_context(tc.tile_pool(name="ps", bufs=2, space="PSUM"))

        # ---- constants (loaded once) ----
        wt_sb = const_pool.tile([128, K * C], BF16)
        nc.sync.dma_start(wt_sb[:], wt[:])
        bt_sb = const_pool.tile([128, C], F32)
        nc.sync.dma_start(bt_sb[:], btile[:])
        kc_sb = const_pool.tile([128, NS * K], F32)
        nc.sync.dma_start(kc_sb[:], kcst[:])

        for b in range(BPC):
            x_sb = x_pool.tile([128, L], BF16)
            nc.sync.dma_start(x_sb[:], x_in[b])

            offT = coef_pool.tile([128, NS * K], F32, tag="offT")
            nc.sync.dma_start(offT[0:CH], offs[b])
            mT = coef_pool.tile([128, NS * K], F32, tag="mT")
            nc.sync.dma_start(mT[0:CH], maskp[b])

            # ---- coefficients on VectorE ----
            # hat(p-j) via relu second differences; with p in [0,2] only two
            # relus are needed: A = relu(p-1), Bq = relu(p-2):
            #   u0 = (1-p) + A ; u1 = p - 2A + Bq ; u2 = A - 2Bq ; c_j = u_j*mask
            pcl = coef_pool.tile([128, NS * K], F32, tag="pcl")
            nc.vector.tensor_tensor(pcl[0:CH], offT[0:CH], kc_sb[0:CH], OP.add)
            nc.vector.tensor_scalar(pcl[0:CH], pcl[0:CH], 0.0, 2.0, OP.max, OP.min)
            ra = coef_pool.tile([128, NS * K], F32, tag="ra")
            nc.vector.tensor_scalar(ra[0:CH], pcl[0:CH], -1.0, 0.0, OP.add, OP.max)
            rb = coef_pool.tile([128, NS * K], F32, tag="rb")
            nc.vector.tensor_scalar(rb[0:CH], pcl[0:CH], -2.0, 0.0, OP.add, OP.max)

            u0 = coef_pool.tile([128, NS * K], F32, tag="u0")
            nc.vector.tensor_scalar(u0[0:CH], pcl[0:CH], -1.0, 1.0, OP.mult, OP.add)
            nc.vector.tensor_tensor(u0[0:CH], u0[0:CH], ra[0:CH], OP.add)
            u1 = coef_pool.tile([128, NS * K], F32, tag="u1")
            nc.vector.tensor_scalar(u1[0:CH], ra[0:CH], -2.0, None, OP.mult)
            nc.vector.tensor_tensor(u1[0:CH], u1[0:CH], pcl[0:CH], OP.add)
            nc.vector.tensor_tensor(u1[0:CH], u1[0:CH], rb[0:CH], OP.add)
            u2 = coef_pool.tile([128, NS * K], F32, tag="u2")
            nc.vector.tensor_scalar(u2[0:CH], rb[0:CH], -2.0, None, OP.mult)
            nc.vector.tensor_tensor(u2[0:CH], u2[0:CH], ra[0:CH], OP.add)
            cj = []
            for j, uj in enumerate((u0, u1, u2)):
                cjt = coef_pool.tile([128, NS * K], F32, tag=f"c{j}")
                nc.vector.tensor_tensor(cjt[0:CH], uj[0:CH], mT[0:CH], OP.mult)
                cj.append(cjt)

            # ---- conv + combine, chunk by chunk ----
            for s in range(NS):
                t0 = s * CH
                ts_ = min(CH, LOUT - t0)     # valid outputs in this chunk

                psj = []
                for j in range(3):
                    msj = min(128, L - (t0 + j))
                    ps = psum_pool.tile([128, K * C], F32, tag=f"ps{j}",
                                        bufs=3 if j < 2 else 2)
                    nc.tensor.matmul(
                        ps[0:msj, :],
                        x_sb[:, t0 + j:t0 + j + msj],
                        wt_sb[:, :],
                        start=True,
                        stop=True,
                    )
                    psj.append(ps)

                col = lambda kk, jj: cj[jj][0:ts_, s * K + kk:s * K + kk + 1]

                # chain A (VectorE): bias + 5 fused terms, fp32
                chainA = [(0, 0), (1, 0), (2, 0), (0, 1), (1, 1)]
                chainB = [(2, 1), (0, 2), (1, 2), (2, 2)]

                accA = [acc_pool.tile([128, C], F32, name="accA0", tag="accA0"),
                        acc_pool.tile([128, C], F32, name="accA1", tag="accA1")]
                prev = bt_sb
                for i, (k, j) in enumerate(chainA):
                    dst = accA[i % 2]
                    nc.vector.scalar_tensor_tensor(
                        dst[0:ts_, :],
                        psj[j][0:ts_, k * C:(k + 1) * C],
                        col(k, j),
                        prev[0:ts_, :],
                        OP.mult,
                        OP.add,
                    )
                    prev = dst

                # chain B: ScalarE scaled copies (fused multiply) into one
                # wide tile; GpSimd folds it with 2 adds (FD=256 then 128)
                tk4 = y_pool.tile([128, 4 * C], BF16, name="tk4", tag="tk4")
                for i, (k, j) in enumerate(chainB):
                    nc.scalar.activation(
                        tk4[0:ts_, i * C:(i + 1) * C],
                        psj[j][0:ts_, k * C:(k + 1) * C],
                        ACTF.Copy,
                        scale=col(k, j),
                    )
                bp = acc_pool.tile([128, 2 * C], BF16, name="bp", tag="bp")
                nc.gpsimd.tensor_tensor(
                    bp[0:ts_, :], tk4[0:ts_, 0:2 * C], tk4[0:ts_, 2 * C:4 * C], OP.add
                )
                b03 = acc_pool.tile([128, C], BF16, name="b03", tag="b03")
                nc.gpsimd.tensor_tensor(
                    b03[0:ts_, :], bp[0:ts_, 0:C], bp[0:ts_, C:2 * C], OP.add
                )
                acc_f = acc_pool.tile([128, C], F32, tag="accF")
                nc.gpsimd.tensor_tensor(
                    acc_f[0:ts_, :], prev[0:ts_, :], b03[0:ts_, :], OP.add
                )
                nc.sync.dma_start(outT[b, t0:t0 + ts_, :], acc_f[0:ts_, :])

    nc.compile()
    _CACHE["nc"] = nc
    return nc


def _make_in_maps(x, offsets, mask, weight, bias):
    x = np.asarray(x, dtype=np.float32)
    offsets = np.asarray(offsets, dtype=np.float32)
    mask = np.asarray(mask, dtype=np.float32)
    weight = np.asarray(weight, dtype=np.float32)
    bias = np.asarray(bias, dtype=np.float32)

    bf16 = ml_dtypes.bfloat16
    x_bf = np.ascontiguousarray(x.astype(bf16))
    # wt[c, k*C + o] = weight[o, c, k]
    wt = np.ascontiguousarray(
        weight.transpose(1, 2, 0).reshape(C, K * C).astype(bf16)
    )
    btile = np.ascontiguousarray(np.broadcast_to(bias[None, :], (128, C)))
    kc = np.tile(np.arange(K, dtype=np.float32), NS)
    kcst = np.ascontiguousarray(np.broadcast_to(kc[None, :], (128, NS * K)))

    # coef staging: [CH, (s, k)] with t = s*CH + p
    offs_pad = np.zeros((B, LPAD, K), np.float32)
    offs_pad[:, :LOUT] = offsets[:, 0]
    offs_pre = np.ascontiguousarray(
        offs_pad.reshape(B, NS, CH, K).transpose(0, 2, 1, 3).reshape(B, CH, NS * K)
    )
    mask_pad = np.zeros((B, K, LPAD), np.float32)
    mask_pad[:, :, :LOUT] = mask
    mask_pre = np.ascontiguousarray(
        mask_pad.reshape(B, K, NS, CH).transpose(0, 3, 2, 1).reshape(B, CH, NS * K)
    )

    in_maps = []
    for cid in range(NCORES):
        sl = slice(cid * BPC, (cid + 1) * BPC)
        in_maps.append({
            "x_in": np.ascontiguousarray(x_bf[sl]),
            "offs": np.ascontiguousarray(offs_pre[sl]),
            "maskp": np.ascontiguousarray(mask_pre[sl]),
            "wt": wt,
            "btile": btile,
            "kcst": kcst,
        })
    return in_maps


def kernel(x, offsets, mask, weight, bias):
    nc = _build_program()
    in_maps = _make_in_maps(x, offsets, mask, weight, bias)
    res = bass_utils.run_bass_kernel_spmd(nc, in_maps, core_ids=list(range(NCORES)))
    out = np.empty((B, C, LOUT), np.float32)
    for cid in range(NCORES):
        out[cid * BPC:(cid + 1) * BPC] = res.results[cid]["outT"].transpose(0, 2, 1)
    return out
